# revision 4
# baseline (speedup 1.0000x reference)
"""AdvancedFeatureGNN (4-layer GCN + pooling + MLP head) on 8 Trainium2 cores.

Sharding (graph/data parallel per the hint):
- Graphs split into 8 contiguous blocks of G/8; each core owns the block's
  nodes (batch is sorted) and the edges whose destination lands there, so
  pooling is core-local.  The small 128-wide weights are replicated.
- Per layer each core gathers h[src] rows for its edges via indirect DMA and
  contracts them on the tensor engine against one-hot selection matrices
  built on-device ((iota == dst_local) * norm) into PSUM, giving
  (A_hat @ h)^T per 128-node tile; then W + folded BatchNorm + ReLU, all in
  [feat, node] orientation so BN is a per-partition affine.  Self loops use a
  contiguous row load and a diagonal matmul instead of gathers.
- After layers 0-2 the per-core slab is AllGather'd so every core holds full
  h for the next layer's gathers.  Layer 3 writes h^T into a
  graph-slot-strided SBUF buffer; sum/mean/max pooling are fixed-slice
  reductions, followed by the fused MLP head on [*, G/8] tiles.

The compiled program is identical on all 8 cores (SPMD); every per-core
difference is carried by input data (index/selection tables).
"""

import numpy as np

P = 128
NC = 8
EPS = 1e-5


def _round_up(a, m):
    return (a + m - 1) // m * m


# ---------------------------------------------------------------- host prep

def _prep(x, graph_features, edge_index, batch,
          conv_W, conv_b, bn_g, bn_b, bn_m, bn_v,
          gm_W1, gm_b1, gm_g1, gm_be1, gm_m1, gm_v1,
          gm_W2, gm_b2, gm_g2, gm_be2, gm_m2, gm_v2,
          fu_W1, fu_b1, fu_g1, fu_be1, fu_m1, fu_v1,
          fu_W2, fu_b2, fu_g2, fu_be2, fu_m2, fu_v2,
          fu_W3, fu_b3):
    f32, i32 = np.float32, np.int32
    x = np.asarray(x, dtype=f32)
    graph_features = np.asarray(graph_features, dtype=f32)
    N, H = x.shape
    G = graph_features.shape[0]
    L = int(np.asarray(conv_W).shape[0])
    GPC = G // NC
    batch = np.asarray(batch).astype(np.int64)
    src = np.asarray(edge_index[0]).astype(np.int64)
    dst = np.asarray(edge_index[1]).astype(np.int64)

    deg = np.bincount(dst, minlength=N).astype(f32) + 1.0
    dis = (1.0 / np.sqrt(deg)).astype(f32)
    dis2 = dis * dis

    gstart = np.searchsorted(batch, np.arange(G + 1))
    core_start = gstart[np.arange(NC + 1) * GPC]
    n_c = np.diff(core_start)
    MAXL = int(_round_up(max(1, int(n_c.max())), P))
    T_c = MAXL // P

    cnt = np.zeros((NC, GPC), np.int64)
    for c in range(NC):
        cnt[c] = np.diff(gstart[c * GPC:(c + 1) * GPC + 1])
    S = np.maximum(32, _round_up(cnt.max(axis=0), 32)).astype(np.int64)
    offs = np.concatenate([[0], np.cumsum(S)])
    TOT3 = int(_round_up(int(offs[-1]), P))
    T_3 = TOT3 // P

    node_ids = np.arange(N)
    node_owner = np.searchsorted(core_start[1:], node_ids, side="right")
    node_local = node_ids - core_start[node_owner]
    node_remap = (node_owner * MAXL + node_local).astype(i32)
    j_of = batch - node_owner * GPC
    within = node_ids - gstart[batch]
    node_scol = offs[j_of] + within

    e_owner = node_owner[dst]
    norm_e = (dis[src] * dis[dst]).astype(f32)

    def build_tables(colmap_dst, srcvals_list, T):
        """Per-core edges grouped by dst tile, packed into 128-slot chunks.

        Chunk counts per tile are maxed across cores so the compiled program
        is identical on every core; shortfalls are padded with norm=0 slots.
        """
        percore = []
        for c in range(NC):
            m = e_owner == c
            cols = colmap_dst[dst[m]]
            order = np.argsort(cols >> 7, kind="stable")
            percore.append((m, order, (cols >> 7)[order], cols[order]))
        Kt = np.zeros(T, np.int64)
        for c in range(NC):
            tiles_s = percore[c][2]
            tc_counts = np.bincount(tiles_s, minlength=T)
            Kt = np.maximum(Kt, (tc_counts + P - 1) // P)
        Kt = np.maximum(Kt, 1)
        base = np.concatenate([[0], np.cumsum(Kt)])
        C = int(base[-1])
        tabs = []
        for c in range(NC):
            m, order, tiles_s, cols_s = percore[c]
            tc_counts = np.bincount(tiles_s, minlength=T)
            tile_first = np.concatenate([[0], np.cumsum(tc_counts)])
            slot = np.arange(len(tiles_s)) - tile_first[tiles_s]
            chunk = base[tiles_s] + (slot >> 7)
            part = slot & 127
            srcs = []
            for sv in srcvals_list:
                t_ = np.zeros((P, C), sv.dtype)
                t_[part, chunk] = sv[m][order]
                srcs.append(t_)
            t_dl = np.zeros((P, C), f32)
            t_dl[part, chunk] = (cols_s & 127).astype(f32)
            t_nm = np.zeros((P, C), f32)
            t_nm[part, chunk] = norm_e[m][order]
            tabs.append((srcs, t_dl, t_nm))
        return Kt, base, C, tabs

    Kt_c, base_c, C0, tabs_c = build_tables(
        node_local, [src.astype(i32), node_remap[src]], T_c)
    Kt_3, base_3, C3, tabs_3 = build_tables(
        node_scol, [node_remap[src]], T_3)

    dis2c = np.zeros((NC, P, T_c), f32)
    xloc = np.zeros((NC, MAXL, H), f32)
    dis2s = np.zeros((NC, P, T_3), f32)
    selfidx3 = np.zeros((NC, P, T_3), i32)
    mask3 = np.zeros((NC, 1, TOT3), f32)
    countinv = np.zeros((NC, 1, GPC), f32)
    gfT0 = np.zeros((NC, graph_features.shape[1], GPC), f32)
    for c in range(NC):
        nodes = np.arange(core_start[c], core_start[c + 1])
        loc = nodes - core_start[c]
        d2 = np.zeros(T_c * P, f32)
        d2[loc] = dis2[nodes]
        dis2c[c] = d2.reshape(T_c, P).T
        xloc[c, :len(nodes)] = x[nodes]
        sc = node_scol[nodes]
        d3 = np.zeros(T_3 * P, f32)
        d3[sc] = dis2[nodes]
        dis2s[c] = d3.reshape(T_3, P).T
        si = np.zeros(T_3 * P, i32)
        si[sc] = loc.astype(i32)
        selfidx3[c] = si.reshape(T_3, P).T
        mk = np.zeros(TOT3, f32)
        mk[sc] = 1.0
        mask3[c, 0] = mk
        countinv[c, 0] = 1.0 / np.maximum(cnt[c], 1)
        gfT0[c] = graph_features[c * GPC:(c + 1) * GPC].T

    def fold(gv, bv, mv, vv, add_b):
        gv, bv, mv, vv = (np.asarray(a, dtype=f32) for a in (gv, bv, mv, vv))
        add_b = np.asarray(add_b, dtype=f32)
        s = (gv / np.sqrt(vv + EPS)).astype(f32)
        return s, ((add_b - mv) * s + bv).astype(f32)

    bnscale = np.zeros((P, L), f32)
    bnbias = np.zeros((P, L), f32)
    for l in range(L):
        bnscale[:, l], bnbias[:, l] = fold(bn_g[l], bn_b[l], bn_m[l],
                                           bn_v[l], conv_b[l])
    gm1s, gm1b = fold(gm_g1, gm_be1, gm_m1, gm_v1, gm_b1)
    gm2s, gm2b = fold(gm_g2, gm_be2, gm_m2, gm_v2, gm_b2)
    fu1s, fu1b = fold(fu_g1, fu_be1, fu_m1, fu_v1, fu_b1)
    fu2s, fu2b = fold(fu_g2, fu_be2, fu_m2, fu_v2, fu_b2)

    meta = dict(N=N, H=H, G=G, L=L, GPC=GPC, MAXL=MAXL, T_c=T_c, T_3=T_3,
                TOT3=TOT3, C0=C0, C3=C3, F_G=graph_features.shape[1],
                Kt_c=Kt_c.tolist(), base_c=base_c.tolist(),
                Kt_3=Kt_3.tolist(), base_3=base_3.tolist(),
                S_list=S.tolist(),
                fu_b3=float(np.asarray(fu_b3).reshape(-1)[0]))

    common = dict(
        x_full=x,
        iota=np.tile(np.arange(P, dtype=f32)[None, :], (P, 1)),
        ident=np.eye(P, dtype=f32),
        onesrow=np.ones((1, P), f32),
        convW=np.asarray(conv_W, dtype=f32).transpose(1, 0, 2).reshape(H, L * H),
        bnscale=bnscale, bnbias=bnbias,
        gmW1=np.asarray(gm_W1, dtype=f32),
        gm1sb=np.stack([gm1s, gm1b], axis=1),
        gmW2=np.asarray(gm_W2, dtype=f32),
        gm2sb=np.stack([gm2s, gm2b], axis=1),
        fuW1=np.asarray(fu_W1, dtype=f32).reshape(4, H, 2 * H)
            .transpose(1, 0, 2).reshape(H, 4 * 2 * H),
        fu1sb=np.stack([fu1s, fu1b], axis=1).reshape(2, H, 2)
            .transpose(1, 0, 2).reshape(H, 4),
        fuW2=np.asarray(fu_W2, dtype=f32).reshape(2, H, H)
            .transpose(1, 0, 2).reshape(H, 2 * H),
        fu2sb=np.stack([fu2s, fu2b], axis=1),
        fuW3=np.asarray(fu_W3, dtype=f32),
    )
    in_maps = []
    for c in range(NC):
        (sv_c, dl_c, nm_c) = tabs_c[c]
        (sv_3, dl_3, nm_3) = tabs_3[c]
        m = dict(common)
        m.update(
            x_local=xloc[c],
            idx0=sv_c[0], idxR=sv_c[1], dst0=dl_c, nrm0=nm_c,
            idx3=sv_3[0], dst3=dl_3, nrm3=nm_3,
            dis2c=dis2c[c], dis2s=dis2s[c], selfidx3=selfidx3[c],
            mask3=mask3[c], countinv=countinv[c], gfT0=gfT0[c],
        )
        in_maps.append(m)
    return meta, in_maps


# ---------------------------------------------------------------- device build

def _build(meta):
    import concourse.bacc as bacc
    import concourse.bass as bass
    import concourse.mybir as mybir
    import concourse.tile as tile

    f32, i32 = mybir.dt.float32, mybir.dt.int32
    AF = mybir.ActivationFunctionType
    OP = mybir.AluOpType

    N, H, L = meta["N"], meta["H"], meta["L"]
    GPC, MAXL = meta["GPC"], meta["MAXL"]
    T_c, T_3, TOT3 = meta["T_c"], meta["T_3"], meta["TOT3"]
    C0, C3 = meta["C0"], meta["C3"]
    Kt_c, base_c = meta["Kt_c"], meta["base_c"]
    Kt_3, base_3 = meta["Kt_3"], meta["base_3"]
    F_G = meta["F_G"]
    S_list = meta["S_list"]

    nc = bacc.Bacc("TRN2", target_bir_lowering=False, debug=False,
                   num_devices=NC)

    def inp(name, shape, dt=f32):
        return nc.dram_tensor(name, list(shape), dt, kind="ExternalInput")

    x_full = inp("x_full", [N, H])
    x_local = inp("x_local", [MAXL, H])
    idx0 = inp("idx0", [P, C0], i32)
    idxR = inp("idxR", [P, C0], i32)
    dst0 = inp("dst0", [P, C0])
    nrm0 = inp("nrm0", [P, C0])
    idx3 = inp("idx3", [P, C3], i32)
    dst3 = inp("dst3", [P, C3])
    nrm3 = inp("nrm3", [P, C3])
    dis2c = inp("dis2c", [P, T_c])
    dis2s = inp("dis2s", [P, T_3])
    selfidx3 = inp("selfidx3", [P, T_3], i32)
    mask3_t = inp("mask3", [1, TOT3])
    iota_t = inp("iota", [P, P])
    ident_t = inp("ident", [P, P])
    onesrow_t = inp("onesrow", [1, P])
    convW_t = inp("convW", [H, L * H])
    bnscale_t = inp("bnscale", [P, L])
    bnbias_t = inp("bnbias", [P, L])
    gmW1_t = inp("gmW1", [F_G, H])
    gm1sb_t = inp("gm1sb", [P, 2])
    gmW2_t = inp("gmW2", [H, H])
    gm2sb_t = inp("gm2sb", [P, 2])
    fuW1_t = inp("fuW1", [H, 8 * H])
    fu1sb_t = inp("fu1sb", [H, 4])
    fuW2_t = inp("fuW2", [H, 2 * H])
    fu2sb_t = inp("fu2sb", [P, 2])
    fuW3_t = inp("fuW3", [H, 1])
    countinv_t = inp("countinv", [1, GPC])
    gfT0_t = inp("gfT0", [F_G, GPC])
    out_t = nc.dram_tensor("out", [1, GPC], f32, kind="ExternalOutput")

    with tile.TileContext(nc) as tc:
        with (
            tc.tile_pool(name="const", bufs=1) as cpool,
            tc.tile_pool(name="work", bufs=8) as wpool,
            tc.tile_pool(name="small", bufs=3) as spool,
            tc.tile_pool(name="psA", bufs=2, space="PSUM") as psA,
            tc.tile_pool(name="psB", bufs=2, space="PSUM") as psB,
            tc.tile_pool(name="psC", bufs=2, space="PSUM") as psC,
            tc.tile_pool(name="dram", bufs=1, space="DRAM") as dram,
        ):
            def load_const(t, shape, dt=f32):
                s = cpool.tile(list(shape), dt, tag=t.name, name=t.name + "_s")
                nc.sync.dma_start(s[:], t[:, :])
                return s

            idx0_s = load_const(idx0, [P, C0], i32)
            idxR_s = load_const(idxR, [P, C0], i32)
            dst0_s = load_const(dst0, [P, C0])
            nrm0_s = load_const(nrm0, [P, C0])
            idx3_s = load_const(idx3, [P, C3], i32)
            dst3_s = load_const(dst3, [P, C3])
            nrm3_s = load_const(nrm3, [P, C3])
            dis2c_s = load_const(dis2c, [P, T_c])
            dis2s_s = load_const(dis2s, [P, T_3])
            selfidx3_s = load_const(selfidx3, [P, T_3], i32)
            mask3_s = load_const(mask3_t, [1, TOT3])
            iota_s = load_const(iota_t, [P, P])
            ident_s = load_const(ident_t, [P, P])
            onesrow_s = load_const(onesrow_t, [1, P])
            convW_s = load_const(convW_t, [H, L * H])
            bnscale_s = load_const(bnscale_t, [P, L])
            bnbias_s = load_const(bnbias_t, [P, L])
            gmW1_s = load_const(gmW1_t, [F_G, H])
            gm1sb_s = load_const(gm1sb_t, [P, 2])
            gmW2_s = load_const(gmW2_t, [H, H])
            gm2sb_s = load_const(gm2sb_t, [P, 2])
            fuW1_s = load_const(fuW1_t, [H, 8 * H])
            fu1sb_s = load_const(fu1sb_t, [H, 4])
            fuW2_s = load_const(fuW2_t, [H, 2 * H])
            fu2sb_s = load_const(fu2sb_t, [P, 2])
            fuW3_s = load_const(fuW3_t, [H, 1])
            ci_s = load_const(countinv_t, [1, GPC])
            gf0_s = load_const(gfT0_t, [F_G, GPC])

            hT3_big = cpool.tile([P, TOT3], f32, tag="hT3", name="hT3_big")

            slabs = [dram.tile([MAXL, H], f32, tag=f"slab{l}",
                               name=f"slab{l}") for l in range(3)]
            hcats = [dram.tile([NC * MAXL, H], f32, addr_space="Shared",
                               tag=f"hcat{l}", name=f"hcat{l}")
                     for l in range(3)]

            def agg_tile(t, src_dram, self_src, idx_s, dst_s, nrm_s,
                         dis2_s, Kt, base, strided):
                aggT = psA.tile([P, P], f32, tag="aggT", name="aggT")
                htile = wpool.tile([P, P], f32, tag="htile", name="htile")
                if strided:
                    nc.gpsimd.indirect_dma_start(
                        out=htile[:], out_offset=None, in_=self_src[:, :],
                        in_offset=bass.IndirectOffsetOnAxis(
                            ap=selfidx3_s[:, t:t + 1], axis=0))
                else:
                    nc.sync.dma_start(htile[:],
                                      self_src[t * P:(t + 1) * P, :])
                D = wpool.tile([P, P], f32, tag="D", name="D")
                nc.vector.tensor_scalar_mul(D[:], ident_s[:],
                                            dis2_s[:, t:t + 1])
                K = Kt[t]
                nc.tensor.matmul(aggT[:], lhsT=htile[:], rhs=D[:],
                                 start=True, stop=(K == 0))
                for k in range(K):
                    col = base[t] + k
                    msgs = wpool.tile([P, P], f32, tag="msgs", name="msgs")
                    nc.gpsimd.indirect_dma_start(
                        out=msgs[:], out_offset=None, in_=src_dram[:, :],
                        in_offset=bass.IndirectOffsetOnAxis(
                            ap=idx_s[:, col:col + 1], axis=0))
                    Pm = wpool.tile([P, P], f32, tag="Pm", name="Pm")
                    nc.vector.tensor_scalar(
                        out=Pm[:], in0=iota_s[:],
                        scalar1=dst_s[:, col:col + 1],
                        scalar2=nrm_s[:, col:col + 1],
                        op0=OP.is_equal, op1=OP.mult)
                    nc.tensor.matmul(aggT[:], lhsT=msgs[:], rhs=Pm[:],
                                     start=False, stop=(k == K - 1))
                return aggT

            def w_matmul(aggT, layer):
                aggT_s = spool.tile([P, P], f32, tag="aggTs", name="aggTs")
                nc.scalar.copy(aggT_s[:], aggT[:])
                ps2 = psB.tile([P, P], f32, tag="ps2", name="ps2")
                nc.tensor.matmul(ps2[:],
                                 lhsT=convW_s[:, layer * H:(layer + 1) * H],
                                 rhs=aggT_s[:], start=True, stop=True)
                return ps2

            # ---------------- layers 0..2 (compact layout)
            for layer in range(3):
                src_dram = x_full if layer == 0 else hcats[layer - 1]
                self_src = x_local if layer == 0 else slabs[layer - 1]
                idx_s = idx0_s if layer == 0 else idxR_s
                for t in range(T_c):
                    aggT = agg_tile(t, src_dram, self_src, idx_s,
                                    dst0_s, nrm0_s, dis2c_s, Kt_c, base_c,
                                    strided=False)
                    ps2 = w_matmul(aggT, layer)
                    hT = spool.tile([P, P], f32, tag="hT", name="hT")
                    nc.scalar.activation(
                        hT[:], ps2[:], AF.Relu,
                        bias=bnbias_s[:, layer:layer + 1],
                        scale=bnscale_s[:, layer:layer + 1])
                    ps3 = psC.tile([P, P], f32, tag="ps3", name="ps3")
                    nc.tensor.transpose(ps3[:], hT[:], ident_s[:])
                    hrow = spool.tile([P, P], f32, tag="hrow", name="hrow")
                    nc.scalar.copy(hrow[:], ps3[:])
                    nc.sync.dma_start(slabs[layer][t * P:(t + 1) * P, :],
                                      hrow[:])
                nc.gpsimd.collective_compute(
                    "AllGather", mybir.AluOpType.bypass,
                    replica_groups=[list(range(NC))],
                    ins=[slabs[layer][:, :]], outs=[hcats[layer][:, :]])

            # ---------------- layer 3 (strided layout, stays in SBUF)
            for t in range(T_3):
                aggT = agg_tile(t, hcats[2], slabs[2], idx3_s,
                                dst3_s, nrm3_s, dis2s_s, Kt_3, base_3,
                                strided=True)
                ps2 = w_matmul(aggT, 3)
                nc.scalar.activation(
                    hT3_big[:, t * P:(t + 1) * P], ps2[:], AF.Relu,
                    bias=bnbias_s[:, 3:4], scale=bnscale_s[:, 3:4])
                psm = psC.tile([P, P], f32, tag="ps3", name="psm")
                nc.tensor.matmul(psm[:], lhsT=onesrow_s[:, :],
                                 rhs=mask3_s[:, t * P:(t + 1) * P],
                                 start=True, stop=True)
                nc.vector.tensor_tensor(
                    out=hT3_big[:, t * P:(t + 1) * P],
                    in0=hT3_big[:, t * P:(t + 1) * P],
                    in1=psm[:], op=OP.mult)

            # ---------------- pooling (fixed slices, uniform across cores)
            sT = cpool.tile([P, GPC], f32, tag="sT", name="sT")
            mxT = cpool.tile([P, GPC], f32, tag="mxT", name="mxT")
            off = 0
            for j in range(GPC):
                w = S_list[j]
                nc.vector.reduce_sum(out=sT[:, j:j + 1],
                                     in_=hT3_big[:, off:off + w],
                                     axis=mybir.AxisListType.X)
                nc.vector.reduce_max(out=mxT[:, j:j + 1],
                                     in_=hT3_big[:, off:off + w],
                                     axis=mybir.AxisListType.X)
                off += w
            psci = psC.tile([P, GPC], f32, tag="ps3", name="psci")
            nc.tensor.matmul(psci[:], lhsT=onesrow_s[:, :], rhs=ci_s[:],
                             start=True, stop=True)
            meanT = cpool.tile([P, GPC], f32, tag="meanT", name="meanT")
            nc.vector.tensor_tensor(out=meanT[:], in0=sT[:], in1=psci[:],
                                    op=OP.mult)

            # ---------------- graph-feature MLP
            psg = psB.tile([P, GPC], f32, tag="ps2", name="psg")
            nc.tensor.matmul(psg[:], lhsT=gmW1_s[:], rhs=gf0_s[:],
                             start=True, stop=True)
            g1 = cpool.tile([P, GPC], f32, tag="g1", name="g1")
            nc.scalar.activation(g1[:], psg[:], AF.Relu,
                                 bias=gm1sb_s[:, 1:2], scale=gm1sb_s[:, 0:1])
            psg2 = psB.tile([P, GPC], f32, tag="ps2", name="psg2")
            nc.tensor.matmul(psg2[:], lhsT=gmW2_s[:], rhs=g1[:],
                             start=True, stop=True)
            gfT = cpool.tile([P, GPC], f32, tag="gfT", name="gfT")
            nc.scalar.activation(gfT[:], psg2[:], AF.Relu,
                                 bias=gm2sb_s[:, 1:2], scale=gm2sb_s[:, 0:1])

            # ---------------- fused head: f = [mean | max | sum | gf]
            blocks = [meanT, mxT, sT, gfT]
            f1 = []
            for h in range(2):
                psf = psB.tile([P, GPC], f32, tag="ps2", name="psf")
                for b in range(4):
                    nc.tensor.matmul(
                        psf[:],
                        lhsT=fuW1_s[:, b * 2 * P + h * P:
                                    b * 2 * P + (h + 1) * P],
                        rhs=blocks[b][:], start=(b == 0), stop=(b == 3))
                f1h = cpool.tile([P, GPC], f32, tag=f"f1_{h}", name=f"f1_{h}")
                nc.scalar.activation(
                    f1h[:], psf[:], AF.Relu,
                    bias=fu1sb_s[:, 2 * h + 1:2 * h + 2],
                    scale=fu1sb_s[:, 2 * h:2 * h + 1])
                f1.append(f1h)
            psf2 = psB.tile([P, GPC], f32, tag="ps2", name="psf2")
            for h in range(2):
                nc.tensor.matmul(psf2[:], lhsT=fuW2_s[:, h * P:(h + 1) * P],
                                 rhs=f1[h][:], start=(h == 0), stop=(h == 1))
            f2 = cpool.tile([P, GPC], f32, tag="f2", name="f2")
            nc.scalar.activation(f2[:], psf2[:], AF.Relu,
                                 bias=fu2sb_s[:, 1:2], scale=fu2sb_s[:, 0:1])
            pso = psB.tile([1, GPC], f32, tag="ps2", name="pso")
            nc.tensor.matmul(pso[:], lhsT=fuW3_s[:], rhs=f2[:],
                             start=True, stop=True)
            ores = cpool.tile([1, GPC], f32, tag="ores", name="ores")
            nc.scalar.activation(ores[:], pso[:], AF.Copy,
                                 bias=float(meta["fu_b3"]), scale=1.0)
            nc.sync.dma_start(out_t[:, :], ores[:])

    nc.compile()
    return nc


# ---------------------------------------------------------------- entry point

def kernel(**inputs):
    meta, in_maps = _prep(**inputs)
    nc = _build(meta)
    from concourse.bass_utils import run_bass_kernel_spmd
    res = run_bass_kernel_spmd(nc, in_maps, core_ids=list(range(NC)))
    outs = [np.asarray(res.results[c]["out"]).reshape(-1) for c in range(NC)]
    return np.concatenate(outs).reshape(-1, 1).astype(np.float32)
